# revision 10
# baseline (speedup 1.0000x reference)
import re
import sys
from contextlib import ExitStack

import numpy as np

try:
    import concourse  # noqa
except ImportError:
    sys.path.insert(0, "/opt/trn_rl_repo")

import ml_dtypes
import concourse.bass as bass
import concourse.dve_ops as dve_ops
import concourse.tile as tile
from concourse import mybir
from concourse.bass_utils import run_bass_kernel_spmd
from concourse.dve_ops import DveOp
from concourse.dve_spec import C0, C1, Spec, Src0, Src1
from concourse.dve_table_gen import dve_ver_for
from concourse.bacc import Bacc

N_CORES = 8
B = 8192
BC = B // N_CORES  # 1024 batch per core
D_IN = 784
KT = 7  # 784 -> 7 k-tiles of 128
D_IN_PAD = KT * 128  # 896
D_H = 1000
HT = 8  # 1000 -> 8 h-tiles of 128
D_H_PAD = HT * 128  # 1024
D_OUT = 10
T = 25
BETA = 0.95
THR = 1.0
NH = 512  # batch half
NTERMS = 3  # exact bf16 decomposition of W2

LAST_EXEC_NS = None
TRACE = False

_CACHE = {}


def _install_ntff_hook():
    try:
        import antenv.axon_hooks  # noqa

        return
    except ImportError:
        pass
    try:
        import types

        import antenv

        mod = types.ModuleType("antenv.axon_hooks")
        mod._hook = None

        def set_axon_ntff_profile_hook(h):
            mod._hook = h

        def get_axon_ntff_profile_hook():
            return mod._hook

        mod.set_axon_ntff_profile_hook = set_axon_ntff_profile_hook
        mod.get_axon_ntff_profile_hook = get_axon_ntff_profile_hook
        sys.modules["antenv.axon_hooks"] = mod
        antenv.axon_hooks = mod
        try:
            from trn_agent_boot.trn_boot import _ntff_profile_via_ctypes

            hook = _ntff_profile_via_ctypes("/opt/axon/libaxon_pjrt.so")
            if hook is not None:
                mod._hook = hook
        except Exception:
            pass
    except Exception:
        pass


def _register_memupd():
    for op in dve_ops.OPS:
        if op.name == "SNN_MEMUPD":
            return op
    spec = Spec(
        body=Src0 * C0 + Src1 - (Src0 > C1),
        reference=lambda in0, in1, s0, s1, imm2: in0 * s0
        + in1
        - (in0 > s1).astype(np.float32),
    )
    op = DveOp("SNN_MEMUPD", spec, subdim=False, uops_sha={})
    dve_ops.OPS.append(op)
    dve_ops.CUSTOM_DVE_SPECS[op.name] = op.spec
    dve_ops._SUB_OPCODE_FOR_NAME[op.name] = (
        dve_ops._CUSTOM_DVE_ROW_BASE + len(dve_ops.OPS) - 1
    )
    ver = dve_ver_for("TRN2")
    try:
        op.compile(ver)
    except ValueError as e:
        m = re.search(r'uops_sha\["%s"\]="([0-9a-f]+)"' % ver, str(e))
        if not m:
            raise
        op.uops_sha[ver] = m.group(1)
        op.compile(ver)
    return op


def _build():
    MEMUPD = _register_memupd()
    nc = Bacc()
    f32 = mybir.dt.float32
    bf16 = mybir.dt.bfloat16
    AF = mybir.ActivationFunctionType
    NS = HT * NTERMS

    xT_d = nc.declare_dram_parameter("xT", [KT, 128, BC], f32, isOutput=False)
    w1T_d = nc.declare_dram_parameter("w1T", [KT, 128, D_H_PAD], f32, isOutput=False)
    b1r_d = nc.declare_dram_parameter("b1r", [128, HT], f32, isOutput=False)
    w2p_d = nc.declare_dram_parameter("w2p", [128, NS, D_OUT], bf16, isOutput=False)
    b2p_d = nc.declare_dram_parameter("b2p", [D_OUT, 1], f32, isOutput=False)
    memT_d = nc.declare_dram_parameter("memT", [T, D_OUT, BC], f32, isOutput=True)

    with tile.TileContext(nc) as tc, ExitStack() as ctx:
        pool = ctx.enter_context(tc.tile_pool(name="sb", bufs=1))
        ppool = ctx.enter_context(tc.tile_pool(name="ps", bufs=1, space="PSUM"))

        xsb = pool.tile([128, KT, BC], f32)
        w1sb = pool.tile([128, KT, D_H_PAD], f32)
        cur1 = pool.tile([128, HT, BC], f32)
        mem1 = pool.tile([128, HT, BC], f32)
        sgn1 = pool.tile([128, HT, BC], bf16)
        w2sb = pool.tile([128, NS, D_OUT], bf16)
        b1sb = pool.tile([128, HT], f32)
        b2sb = pool.tile([D_OUT, 1], f32)
        negone = pool.tile([128, 1], f32)
        mem2 = pool.tile([D_OUT, BC], f32)
        cur2 = pool.tile([D_OUT, BC], f32)

        p1a = ppool.tile([128, NH], f32)
        p1b = ppool.tile([128, NH], f32)
        p2a = ppool.tile([D_OUT, NH], f32)
        p2b = ppool.tile([D_OUT, NH], f32)

        nc.gpsimd.memset(negone[:], -1.0)

        # DMA order tuned so PE can start group (b=0, h=0) early.
        nc.sync.dma_start(b1sb[:], b1r_d[:])
        nc.sync.dma_start(b2sb[:], b2p_d[:])
        nc.sync.dma_start(w2sb[:], w2p_d[:])
        for k in range(KT):
            nc.sync.dma_start(xsb[:, k, 0:NH], xT_d[k, :, 0:NH])
            nc.sync.dma_start(w1sb[:, k, 0:128], w1T_d[k, :, 0:128])
        for h in range(1, HT):
            for k in range(KT):
                nc.sync.dma_start(
                    w1sb[:, k, 128 * h : 128 * (h + 1)],
                    w1T_d[k, :, 128 * h : 128 * (h + 1)],
                )
        for k in range(KT):
            nc.sync.dma_start(xsb[:, k, NH:BC], xT_d[k, :, NH:BC])

        # fc1: cur1[:, h, b*NH:] = x @ W1.T + b1  (fp32 PE, PSUM k-accum)
        for b in range(2):
            bs = slice(b * NH, (b + 1) * NH)
            for h in range(HT):
                pt = p1a if (b * HT + h) % 2 == 0 else p1b
                for k in range(KT):
                    nc.tensor.matmul(
                        pt[:],
                        w1sb[:, k, 128 * h : 128 * (h + 1)],
                        xsb[:, k, bs],
                        start=(k == 0),
                        stop=(k == KT - 1),
                    )
                nc.scalar.activation(
                    cur1[:, h, bs], pt[:], AF.Identity, bias=b1sb[:, h : h + 1]
                )

        def fc2_half(pt, hb, out_ap):
            bs = slice(hb * NH, (hb + 1) * NH)
            for h in range(HT):
                for ti in range(NTERMS):
                    s = h * NTERMS + ti
                    nc.tensor.matmul(
                        pt[:],
                        w2sb[:, s, :],
                        sgn1[:, h, bs],
                        start=(s == 0),
                        stop=(s == NS - 1),
                    )
            nc.scalar.activation(out_ap, pt[:], AF.Identity, bias=b2sb[:])

        # t = 0: mem1 = cur1 (implicit), sgn1 = Sign(cur1 - 1), mem2 = cur2
        for hb in range(2):
            bs = slice(hb * NH, (hb + 1) * NH)
            nc.scalar.activation(
                sgn1[:, :, bs], cur1[:, :, bs], AF.Sign, bias=negone[:]
            )
            fc2_half(p2a if hb == 0 else p2b, hb, mem2[:, bs])
        nc.sync.dma_start(memT_d[0], mem2[:])

        # steps 1..24, layer-2 finalize lags one step on DVE
        for t in range(1, T):
            src1 = cur1 if t == 1 else mem1
            for hb in range(2):
                bs = slice(hb * NH, (hb + 1) * NH)
                nc.vector._custom_dve(
                    MEMUPD,
                    out=mem1[:, :, bs],
                    in0=src1[:, :, bs],
                    in1=cur1[:, :, bs],
                    s0=BETA,
                    s1=THR,
                )
            if t >= 2:
                nc.vector._custom_dve(
                    MEMUPD, out=mem2[:], in0=mem2[:], in1=cur2[:], s0=BETA, s1=THR
                )
                nc.sync.dma_start(memT_d[t - 1], mem2[:])
            for hb in range(2):
                bs = slice(hb * NH, (hb + 1) * NH)
                nc.scalar.activation(
                    sgn1[:, :, bs], mem1[:, :, bs], AF.Sign, bias=negone[:]
                )
                fc2_half(p2a if hb == 0 else p2b, hb, cur2[:, bs])
        nc.vector._custom_dve(
            MEMUPD, out=mem2[:], in0=mem2[:], in1=cur2[:], s0=BETA, s1=THR
        )
        nc.sync.dma_start(memT_d[T - 1], mem2[:])

    nc.finalize()
    return nc


def _prep_shared(W1, b1, W2, b2):
    bf = ml_dtypes.bfloat16
    w1T = np.zeros((KT * 128, D_H_PAD), np.float32)
    w1T[:D_IN, :D_H] = W1.T
    w1T = np.ascontiguousarray(w1T.reshape(KT, 128, D_H_PAD))

    b1pad = np.zeros(D_H_PAD, np.float32)
    b1pad[:D_H] = b1
    b1r = np.ascontiguousarray(b1pad.reshape(HT, 128).T)

    w2pad = np.zeros((D_OUT, D_H_PAD), np.float32)
    w2pad[:, :D_H] = W2
    terms = []
    r = w2pad.copy()
    for _ in range(NTERMS):
        tb = r.astype(bf)
        terms.append(tb)
        r = r - tb.astype(np.float32)
    w2p = np.zeros((128, HT * NTERMS, D_OUT), bf)
    for h in range(HT):
        for ti, tb in enumerate(terms):
            half = (0.5 * tb[:, 128 * h : 128 * (h + 1)].astype(np.float32)).astype(bf)
            w2p[:, h * NTERMS + ti, :] = half.T

    b2p = (b2.astype(np.float64) + 0.5 * w2pad.astype(np.float64).sum(axis=1)).astype(
        np.float32
    )
    return w1T, b1r, w2p, b2p.reshape(D_OUT, 1)


def kernel(**inputs):
    global LAST_EXEC_NS
    x = np.ascontiguousarray(np.asarray(inputs["x"], dtype=np.float32))
    W1 = np.asarray(inputs["W1"], dtype=np.float32)
    b1 = np.asarray(inputs["b1"], dtype=np.float32)
    W2 = np.asarray(inputs["W2"], dtype=np.float32)
    b2 = np.asarray(inputs["b2"], dtype=np.float32)

    if "nc" not in _CACHE:
        _CACHE["nc"] = _build()
    nc = _CACHE["nc"]

    w1T, b1r, w2p, b2p = _prep_shared(W1, b1, W2, b2)

    in_maps = []
    for c in range(N_CORES):
        xc = x[c * BC : (c + 1) * BC]  # [BC, 784]
        xT = np.zeros((KT * 128, BC), np.float32)
        xT[:D_IN] = xc.T
        in_maps.append(
            {
                "xT": np.ascontiguousarray(xT.reshape(KT, 128, BC)),
                "w1T": w1T,
                "b1r": b1r,
                "w2p": w2p,
                "b2p": b2p,
            }
        )

    if TRACE:
        _install_ntff_hook()
    br = run_bass_kernel_spmd(nc, in_maps, list(range(N_CORES)), trace=TRACE)
    LAST_EXEC_NS = br.exec_time_ns

    mem2_rec = np.empty((T, B, D_OUT), np.float32)
    for c in range(N_CORES):
        memT = br.results[c]["memT"]  # [T, D_OUT, BC]
        mem2_rec[:, c * BC : (c + 1) * BC, :] = np.transpose(memT, (0, 2, 1))
    spk2_rec = (mem2_rec > THR).astype(np.float32)
    return spk2_rec, mem2_rec
